# revision 6
# baseline (speedup 1.0000x reference)
"""Trainium2 Bass kernel: weighted sliding-window min (STL 'Always' robustness).

out[n, w] = min_k( input[n, 4*w + k] * And_weight[0, k] ),  k in [0, 16)

Strategy (8 NeuronCores, data-parallel over batch N=1024 -> 128 rows/core):
  - Host: cast input to bf16 and deinterleave each row into 4 phase planes
    P_j[b] = x[4b + j]  ([N, 4, B] layout, no padding needed).
  - Device: 16 products p_{o,j} = P_j * c[4o+j] computed FULL-WIDTH, split
    between DVE (tensor_scalar, bf16 4x mode) and ScalarE (ACTIVATE-with-
    scale; wide instructions amortize its fixed overhead). ScalarE's last
    two planes are column-chunked so only the matching merge tile waits.
  - Min tree on DVE (tensor_tensor bf16 2x): Q = min(A, B) over 8 slots
    (Q[0:4] and Q[4:6] full-width; Q[6:8] per merge tile), then per tile
    U = min(Q[0:4], Q[4:8] shifted +2), R = min(U halves), and
    out[w] = min(r0[w], r1[w+1]); per-tile output DMA overlaps compute.
  - Output is written bf16 (exact: a min picks one of the bf16 products)
    and upcast to float32 on the host.
"""

import os as _os

import numpy as np

# Problem geometry (hardcoded; harness calls kernel() with these shapes)
N, L = 1024, 8192
K, S = 16, 4
W = (L - K) // S + 1          # 2045 output windows per row
NCORES = 8
ROWS = N // NCORES            # 128 rows per core == SBUF partitions
B = L // S                    # 2048 blocks of 4 per row


def _parse(s):
    return [int(x) for x in s.split(",") if x != ""]


# Merge tiles over the output-window axis (products are full-width; only the
# min tree + output DMA are tiled). Last tile small -> short drain cascade.
BTS = _parse(_os.environ.get("K_BTS", "512,672,672,189"))
assert sum(BTS) == W, (sum(BTS), W)
NT = len(BTS)
TWS = [bt + 3 for bt in BTS]
BASES = [sum(BTS[:t]) for t in range(NT)]
for _t in range(NT):
    assert BASES[_t] + TWS[_t] <= B

# ScalarE product planes: AFULL full-width planes (in order), then ATILED
# planes emitted per merge tile (chunked so tile t's Q[6:8] merge only waits
# for chunk t). Everything else runs on DVE.
AFULL = int(_os.environ.get("K_AFULL", "4"))
ATILED = int(_os.environ.get("K_ATILED", "4"))
ACT_FULL_ORDER = [(2, 0), (3, 0), (2, 1), (3, 1), (2, 2), (3, 2)]
ACT_TILED_ORDER = [(2, 2), (3, 2), (2, 3), (3, 3)]
# DVE extra products positioned as queue filler between Q[0:4] and Q[4:6].
DVE_LATE_ORDER = [(2, 2), (3, 2), (2, 3), (3, 3)]

_COMPILED = {}


def _plan():
    act_full = ACT_FULL_ORDER[:AFULL]
    act_tiled = [p for p in ACT_TILED_ORDER if p not in act_full][:ATILED]
    dve_late = [p for p in DVE_LATE_ORDER if p not in act_full and p not in act_tiled]
    all_prod = [(o, j) for o in range(4) for j in range(4)]
    dve_early = [
        p
        for p in all_prod
        if p not in act_full and p not in act_tiled and p not in dve_late
    ]
    # early DVE products sorted by plane arrival order (j, then o)
    dve_early.sort(key=lambda p: (p[1], p[0]))
    return act_full, act_tiled, dve_early, dve_late


def _build_bass():
    import concourse.bacc as bacc
    import concourse.mybir as mybir
    from concourse.tile import TileContext

    BF16 = mybir.dt.bfloat16
    F32 = mybir.dt.float32
    MIN = mybir.AluOpType.min

    nc = bacc.Bacc(enable_partition_id=False)
    x = nc.dram_tensor("x", [ROWS, 4 * B], BF16, kind="ExternalInput")
    w = nc.dram_tensor("w", [ROWS, 16], F32, kind="ExternalInput")
    out = nc.dram_tensor("out", [ROWS, W], BF16, kind="ExternalOutput")

    # slot(o, j): plane ordering keeping every min-tree level a dense
    # step-1 access pattern (A holds j-even products, B j-odd):
    #   Q = [q0A q1A q0B q1B | q2A q3A q2B q3B]
    def slot(o, j):
        return 4 * (o // 2) + 2 * (j // 2) + (o % 2)

    act_full, act_tiled, dve_early, dve_late = _plan()

    with TileContext(nc) as tc:
        with (
            tc.tile_pool(name="wp", bufs=1) as wp,
            tc.tile_pool(name="xin", bufs=1) as xin,
            tc.tile_pool(name="pp", bufs=1) as pp,
            tc.tile_pool(name="qq", bufs=1) as qq,
            tc.tile_pool(name="uu", bufs=2) as uu,
            tc.tile_pool(name="rr", bufs=2) as rr,
            tc.tile_pool(name="oo", bufs=2) as oo,
        ):
            # Dummy first Activation so Bacc hoists the ACT table load to the
            # top of the Scalar queue.
            dummy = wp.tile([ROWS, 1], F32)
            nc.scalar.memzero(dummy[:, :])
            w_sb = wp.tile([ROWS, 16], F32)
            nc.scalar.dma_start(out=w_sb[:, :], in_=w[:, :])

            # Serial full-plane DMAs on the sync queue: plane j lands ~1.6us
            # after plane j-1; products are ordered by plane arrival.
            xp = xin.tile([ROWS, 4, B], BF16, name="xp")
            for j in range(4):
                nc.sync.dma_start(out=xp[:, j, :], in_=x[:, j * B : (j + 1) * B])

            A = pp.tile([ROWS, 8, B], BF16, name="A")
            Bb = pp.tile([ROWS, 8, B], BF16, name="Bb")

            def emit_prod(eng, o, j, lo=0, hi=B):
                dst = A if (j % 2 == 0) else Bb
                s = slot(o, j)
                sc = w_sb[:, 4 * o + j : 4 * o + j + 1]
                if eng == "act":
                    nc.scalar.mul(
                        out=dst[:, s, lo:hi], in_=xp[:, j, lo:hi], mul=sc
                    )
                else:
                    nc.vector.tensor_scalar_mul(
                        out=dst[:, s, lo:hi], in0=xp[:, j, lo:hi], scalar1=sc
                    )

            # Scalar queue: full-width products, then per-tile chunks of the
            # last planes (chunk t covers merge tile t's columns).
            for (o, j) in act_full:
                emit_prod("act", o, j)
            # Disjoint column chunks: chunk t ends where merge tile t's read
            # range ends, so Q[6:8] of tile t waits only on chunks <= t.
            prev_hi = 0
            for t in range(NT):
                lo = prev_hi
                hi = BASES[t] + TWS[t] if t < NT - 1 else B
                prev_hi = hi
                for (o, j) in act_tiled:
                    emit_prod("act", o, j, lo, hi)

            # DVE queue: early products (arrival order), Q[0:4] full, late
            # products (queue filler while ACT finishes slots 4/5), Q[4:6]
            # full, then per tile: Q[6:8], U, R, O + output DMA.
            Q = qq.tile([ROWS, 8, B], BF16, name="Q")
            for (o, j) in dve_early:
                emit_prod("dve", o, j)
            nc.vector.tensor_tensor(
                out=Q[:, 0:4, :], in0=A[:, 0:4, :], in1=Bb[:, 0:4, :], op=MIN
            )
            for (o, j) in dve_late:
                emit_prod("dve", o, j)
            nc.vector.tensor_tensor(
                out=Q[:, 4:6, :], in0=A[:, 4:6, :], in1=Bb[:, 4:6, :], op=MIN
            )
            for t in range(NT):
                base, tw, bt = BASES[t], TWS[t], BTS[t]
                nc.vector.tensor_tensor(
                    out=Q[:, 6:8, base : base + tw],
                    in0=A[:, 6:8, base : base + tw],
                    in1=Bb[:, 6:8, base : base + tw],
                    op=MIN,
                )
                U = uu.tile([ROWS, 4, tw - 2], BF16, tag="U", name="U")
                nc.vector.tensor_tensor(
                    out=U[:, :, :],
                    in0=Q[:, 0:4, base : base + tw - 2],
                    in1=Q[:, 4:8, base + 2 : base + tw],
                    op=MIN,
                )
                R = rr.tile([ROWS, 2, tw - 2], BF16, tag="R", name="R")
                nc.vector.tensor_tensor(
                    out=R[:, :, :], in0=U[:, 0:2, :], in1=U[:, 2:4, :], op=MIN
                )
                ot = oo.tile([ROWS, bt], BF16, tag="ot", name="ot")
                nc.vector.tensor_tensor(
                    out=ot[:, :], in0=R[:, 0, 0:bt], in1=R[:, 1, 1 : bt + 1], op=MIN
                )
                nc.sync.dma_start(out=out[:, base : base + bt], in_=ot[:, :])
    nc.finalize()
    return nc


def _host_prep(input_f32, And_weight):
    """Shard + relayout host-side. Returns in_maps for the 8 cores."""
    import ml_dtypes

    xb = np.asarray(input_f32, dtype=np.float32).astype(ml_dtypes.bfloat16)
    # [N, L] -> [N, B, 4] -> [N, 4, B] phase planes, flattened per row
    planes = np.ascontiguousarray(xb.reshape(N, B, S).transpose(0, 2, 1))
    xflat = planes.reshape(N, 4 * B)

    wfull = np.broadcast_to(
        np.asarray(And_weight, dtype=np.float32).reshape(1, K), (ROWS, K)
    ).copy()

    in_maps = []
    for c in range(NCORES):
        in_maps.append(
            {
                "x": np.ascontiguousarray(xflat[c * ROWS : (c + 1) * ROWS]),
                "w": wfull,
            }
        )
    return in_maps


def _get_nc():
    if "nc" not in _COMPILED:
        _COMPILED["nc"] = _build_bass()
    return _COMPILED["nc"]


def _run(in_maps, trace=False, **kw):
    from concourse.bass_utils import run_bass_kernel_spmd

    nc = _get_nc()
    res = run_bass_kernel_spmd(
        nc, in_maps, core_ids=list(range(NCORES)), trace=trace, **kw
    )
    return res


def kernel(input, And_weight):
    in_maps = _host_prep(input, And_weight)
    res = _run(in_maps, trace=False)
    out = np.concatenate([res.results[c]["out"] for c in range(NCORES)], axis=0)
    return out.astype(np.float32)
